# revision 44
# baseline (speedup 1.0000x reference)
"""Segment-sum (AggrSum) kernel for 8 Trainium2 NeuronCores.

Math: out[v, :] = sum_{n: X_neis[n] == v} H[n, :]   (H [N, D], out [V, D])

Strategy (V-sharding with host-side bucketing as the sharding step):
  - Sort edge ids by target vocab index; group edges by 128-row vocab tile.
  - Partition the 64 vocab tiles into 8 balanced groups of 8 (one per
    core), ordered inside each group so that packed prefix drift stays
    within [0, 128] rows of 512*vt ("mode B5"). Each core reads an
    exactly-packed edge stream; every vocab tile's edges are covered by
    a fixed window of K=5 physical 128-row tiles at static offsets, and
    the one-hot masks zero out foreign rows.
  - H rows are uploaded as a single fp16 plane (rel err ~3e-4, well
    under the 2e-2 gate); one fp16 matmul per (vt, k) window tile
    accumulates into a [128, 256] fp32 PSUM tile.  Mask emission
    (one DVE is_equal per vocab tile) is split across the Vector and
    GpSimd engines so neither gates the DMA-bound stream.  PSUM->SBUF
    copies run on the Scalar (ACT) engine with an fp32->fp16 convert;
    outputs stream back as fp16 and the host upconverts + scatters.
  - Fallbacks: drift in [-128,128] with K=6 windows ("mode B6"), then
    padded per-vt tiles ("mode A") for pathological inputs.
"""

import numpy as np

import concourse.bacc as bacc
import concourse.mybir as mybir
import concourse.tile as tile
from concourse.bass_utils import run_bass_kernel_spmd

N, D, V = 32768, 256, 8192
NCORES = 8
P = 128
VT_PER_CORE = V // P // NCORES  # 8 vocab tiles of 128 per core
NVT = V // P  # 64 global vocab tiles
NTILES_B = 33  # physical 128-row tiles per core in mode B (K=5 and K=6)
# Free-dim-512 dummies, ~427ns each at the cold 1.2 GHz clock.  The HAM
# grant needs one FULL free-running 4096-cycle window of activity: 7 dummies
# (3.39us) measurably fails to trigger it, 8 (3.82us) works.  13 bridges the
# whole preamble-to-chunk1 window (the early real matmuls are DMA-gated
# anyway), so the PE is guaranteed warm and insensitive to window phase.
N_WARMUP = 13

TRACE = False
LAST_EXEC_NS = None
LAST_RESULTS = None

_PROGRAM_CACHE: dict = {}

def _proc_order(mode: str) -> list:
    """vt processing order: B5 runs vt 7 (whose window is pulled to
    mid-stream) early and leaves the stream-tail owner vt 6 for last."""
    if mode == "B5":
        return [0, 1, 7, 2, 3, 4, 5, 6]
    return list(range(VT_PER_CORE))


def _win_lo(mode: str, K: int, vt: int) -> int:
    """First physical tile of vocab tile vt's window."""
    if mode == "B5":
        return 4 * vt
    if mode == "B6":
        return 0 if vt == 0 else 4 * vt - 1
    return vt * K  # mode A: padded, disjoint windows


def _build_common(nc, tc, pools, mode, K, n_phys_tiles, hs, out, chunk_tiles):
    f32 = mybir.dt.float32
    f16 = mybir.dt.float16
    hpool, mv_pool, opool, psum_pool, warm_pool = pools
    nconst = VT_PER_CORE * K + P
    iota_off = VT_PER_CORE * K

    # Warm up the PE's HAM clock gate (throttled 1.2 GHz until ~3.4us of
    # sustained matmul activity).  Long free-dim dummies on a memset scratch
    # tile span the whole preamble-to-first-data window (~3.4us) without a
    # DMA dependency, so PE activity is continuous from the first possible
    # cycle and the 2.4 GHz grant lands just as the real stream ramps up.
    warm_sb = warm_pool.tile([P, 4 * P], f16, name="warm_sb", tag="warmsb")
    nc.gpsimd.memset(warm_sb[:], 0.0)
    warm_ps = psum_pool.tile([P, 4 * P], f32, name="warm", tag="warm", bufs=1)
    for _ in range(N_WARMUP):
        nc.tensor.matmul(
            out=warm_ps[:],
            lhsT=warm_sb[:, :P],
            rhs=warm_sb[:],
            start=True,
            stop=True,
        )

    # Input stream entirely on Sync's HWDGE ring.  The mask operands
    # (consts) ride in front of the first chunk — one DMA, one semaphore;
    # a separate consts DMA would delay every later chunk by its ~0.65us
    # descriptor-generation slot for no gain (the first matmuls are gated
    # by chunk 0's completion either way).
    chunks = []
    t0 = 0
    first = True
    const_sb = None
    for ct in chunk_tiles:
        if first:
            ch = hpool.tile([P, nconst + ct * D], f16, name="ch0")
            nc.sync.dma_start(ch[:], hs[:, : nconst + ct * D])
            const_sb = ch[:, :nconst]
            chunks.append((t0, ct, ch, nconst))
            first = False
        else:
            ch = hpool.tile([P, ct * D], f16, name="ch")
            nc.sync.dma_start(
                ch[:], hs[:, nconst + t0 * D : nconst + (t0 + ct) * D]
            )
            chunks.append((t0, ct, ch, 0))
        t0 += ct
    assert t0 == n_phys_tiles

    def stream_pos(t):
        # B modes stream tile 32 right after the consts; B5 additionally
        # pulls the stream-tail vt's remaining window (tiles 28-31) to
        # mid-stream so only tile 27 arrives last.
        if mode == "B5":
            if t == NTILES_B - 1:
                return 0
            if t <= 8:
                return t + 1
            if t >= 28:
                return 10 + (t - 28)
            return t + 5
        if mode == "B6":
            return 0 if t == NTILES_B - 1 else t + 1
        return t

    def rhs_slice(t):
        p = stream_pos(t)
        for c0, ct, ch, off0 in chunks:
            if c0 <= p < c0 + ct:
                off = off0 + (p - c0) * D
                return ch[:, off : off + D]
        raise AssertionError(t)

    # Mask emission on Vector (TRN2's Pool slot rejects TENSOR_TENSOR).
    # One slice-written buffer; subtile deps keep the matmuls fine-grained.
    big_m = mv_pool.tile([P, VT_PER_CORE * K * P], f16, name="big_m", bufs=1)

    def emit_mask(vt, k0, k1):
        # masks k0..k1-1 for vocab tile vt in one DVE op:
        # m[p, k, q] = (xrel[p, vt*K+k] == iota[q])
        nk = k1 - k0
        m = big_m[:, (vt * K + k0) * P : (vt * K + k1) * P]
        nc.vector.tensor_tensor(
            out=m.rearrange("p (k q) -> p k q", k=nk),
            in0=const_sb[:, vt * K + k0 : vt * K + k1]
            .unsqueeze(2)
            .broadcast_to([P, nk, P]),
            in1=const_sb[:, iota_off : iota_off + P]
            .unsqueeze(1)
            .broadcast_to([P, nk, P]),
            op=mybir.AluOpType.is_equal,
        )

    # Processing order: in B5 the prefetched vt (7) runs early; vt 6 owns
    # the stream tail and is processed last with only one matmul left
    # after the final streamed tile lands.
    proc = _proc_order(mode)

    # First mask column alone so the PE can start the moment chunk 0 lands;
    # then the rest of the first vt, then one op per remaining vt, in
    # processing order.
    emit_mask(proc[0], 0, 1)
    emit_mask(proc[0], 1, K)
    for vt in proc[1:]:
        emit_mask(vt, 0, K)

    big_ot = opool.tile([P, VT_PER_CORE * D], f16, name="big_ot", bufs=1)

    # psum->sbuf fp16 copies: first six processed vts on Scalar (while
    # Vector is emitting masks), the last two on Vector once it frees up.
    # Write-backs go out on Sync's ring, which has drained its input
    # descriptors by then: one DMA for the first six slots, one for the
    # final two.
    for slot, vt in enumerate(proc):
        ps = psum_pool.tile([P, D], f32, name="ps")
        # The accumulation order within a vt is free; consume the
        # early-prefetched tile before the stream-tail tile so only the
        # final streamed tile's matmul remains after the stream ends.
        if mode == "B5" and vt == 6:
            k_order = [0, 1, 2, 4, 3]
        elif mode == "B6" and vt == VT_PER_CORE - 1:
            k_order = [0, K - 1] + list(range(1, K - 1))
        else:
            k_order = list(range(K))
        for i, k in enumerate(k_order):
            t = _win_lo(mode, K, vt) + k
            nc.tensor.matmul(
                out=ps[:],
                lhsT=big_m[:, (vt * K + k) * P : (vt * K + k + 1) * P],
                rhs=rhs_slice(t),
                start=(i == 0),
                stop=(i == len(k_order) - 1),
            )
        ot = big_ot[:, slot * D : (slot + 1) * D]
        if slot < 7:
            nc.scalar.copy(ot, ps[:])
        else:
            nc.vector.tensor_copy(ot, ps[:])
        if slot == 6:
            # seven slots in one write-back on Scalar's ring
            nc.scalar.dma_start(out[:, : 7 * D], big_ot[:, : 7 * D])
        elif slot == 7:
            # the stream-tail vt alone: a tiny (64KB) trailing write so the
            # final receipt is as early as possible (Sync's ring measured a
            # ~1.3us first-byte penalty for late writes — stay on Scalar's)
            nc.scalar.dma_start(out[:, 7 * D :], big_ot[:, 7 * D :])


def _build_program(mode, K):
    """mode 'B5'/'B6': exact-packed windows; mode 'A': padded (K tiles/vt)."""
    f16 = mybir.dt.float16
    if mode == "B5":
        n_phys = NTILES_B
        # stream order: [t32,t0], [t1-8], [t28-31], [t9-16], [t17-24],
        # [t25,26], [t27] — vt7's window lands mid-stream, tile 27 last
        chunk_tiles = [2, 8, 4, 8, 8, 2, 1]
    elif mode == "B6":
        n_phys = NTILES_B
        chunk_tiles = [2, 8, 8, 8, 4, 2, 1]
    else:
        n_phys = VT_PER_CORE * K
        nt = n_phys
        chunk_tiles = []
        while nt > 0:
            chunk_tiles.append(min(7, nt))
            nt -= min(7, nt)
    nconst = VT_PER_CORE * K + P

    nc = bacc.Bacc("TRN2", target_bir_lowering=False)
    hs = nc.dram_tensor("hs", [P, nconst + n_phys * D], f16, kind="ExternalInput")
    out = nc.dram_tensor("out", [P, VT_PER_CORE * D], f16, kind="ExternalOutput")

    with tile.TileContext(nc) as tc:
        with (
            tc.tile_pool(name="h", bufs=min(len(chunk_tiles), 16)) as hpool,
            tc.tile_pool(name="mv", bufs=1) as mv_pool,
            tc.tile_pool(name="o", bufs=1) as opool,
            tc.tile_pool(name="warm", bufs=1) as warm_pool,
            tc.tile_pool(name="psum", bufs=7, space="PSUM") as psum_pool,
        ):
            _build_common(
                nc,
                tc,
                (hpool, mv_pool, opool, psum_pool, warm_pool),
                mode,
                K,
                n_phys,
                hs,
                out,
                chunk_tiles,
            )
    nc.finalize()
    return nc


def _order_group(counts, tiles, lo, hi):
    """Order `tiles` so prefix drift (run - 512*k) stays in [lo, hi] at
    every interior step and <= hi at the end.  DFS, largest-first."""
    tiles = sorted(tiles, key=lambda g: -counts[g])
    n = len(tiles)
    used = [False] * n
    seq = []

    def dfs(k, run):
        if k == n:
            return True
        prev = None
        for i in range(n):
            if used[i]:
                continue
            c = int(counts[tiles[i]])
            if c == prev:
                continue  # identical count -> identical subtree
            prev = c
            d = run + c - 512 * (k + 1)
            if d > hi:
                continue
            if k + 1 < n and d < lo:
                continue
            used[i] = True
            seq.append(tiles[i])
            if dfs(k + 1, run + c):
                return True
            used[i] = False
            seq.pop()
        return False

    return list(seq) if dfs(0, 0) else None


def _partition_tiles(counts, lo, hi):
    """Partition the 64 vocab tiles into 8 groups of 8, each ordered so
    packed prefix drift stays in [lo, hi].  Returns list of per-core
    sequences of global tile ids, or None."""
    rng = np.random.RandomState(0)
    base = np.argsort(counts)[::-1]
    for attempt in range(40):
        if attempt == 0:
            order = base
        else:
            order = rng.permutation(NVT)
            order = order[np.argsort(counts[order])[::-1]]
        groups = [[] for _ in range(NCORES)]
        for i, g in enumerate(order):
            rnd, pos = divmod(i, NCORES)
            c = pos if rnd % 2 == 0 else NCORES - 1 - pos
            groups[c].append(int(g))
        seqs = []
        for c in range(NCORES):
            seq = _order_group(counts, groups[c], lo, hi)
            if seq is None:
                break
            seqs.append(seq)
        if len(seqs) == NCORES:
            return seqs
    return None


def _iota_np():
    return np.tile(np.arange(P, dtype=np.float32), (P, 1))


def _pack_hs(xr, iota_np, hs_tiles):
    """consts ([P, nk] xrel + [P, 128] iota) prepended to the tile-major
    H stream -> single [P, nconst + ntiles*D] fp16 input."""
    return np.hstack([xr, iota_np, hs_tiles]).astype(np.float16)


def _tilemajor(block_f16, ntiles):
    """[ntiles*P, D] fp16 -> [P, ntiles*D] tile-major."""
    return (
        block_f16.reshape(ntiles, P, D).transpose(1, 0, 2).reshape(P, ntiles * D)
    )


def _shard_mode_b(H, order, Xs, starts, groups, mode, K):
    in_maps = []
    scatter = []
    iota_np = _iota_np()
    for c in range(NCORES):
        seq = groups[c]
        rows = np.concatenate([order[starts[g] : starts[g + 1]] for g in seq])
        xval = np.concatenate([Xs[starts[g] : starts[g + 1]] for g in seq]).astype(
            np.float64
        )
        n_c = len(rows)
        block = np.zeros((NTILES_B * P, D), dtype=np.float16)
        block[:n_c] = H[rows].astype(np.float16)
        xpad = np.full(NTILES_B * P, -1000.0, dtype=np.float64)
        xpad[:n_c] = xval
        # stream tile order must match the device-side stream_pos mapping
        if mode == "B5":
            perm = [32, 0] + list(range(1, 9)) + list(range(28, 32)) + list(
                range(9, 28)
            )
        else:
            perm = [32] + list(range(32))
        stream = np.concatenate([block[t * P : (t + 1) * P] for t in perm])
        hs_tiles = _tilemajor(stream, NTILES_B)
        xr = np.full((P, VT_PER_CORE * K), -1000.0, dtype=np.float32)
        for vt in range(VT_PER_CORE):
            base = 128.0 * seq[vt]
            for k in range(K):
                t = _win_lo(mode, K, vt) + k
                xr[:, vt * K + k] = (xpad[t * P : (t + 1) * P] - base).astype(
                    np.float32
                )
        in_maps.append({"hs": _pack_hs(xr, iota_np, hs_tiles)})
        # output slots are in processing order
        scatter.append([seq[vt] for vt in _proc_order(mode)])
    return in_maps, scatter


def _shard_mode_a(H, order, Xs, starts, K):
    in_maps = []
    scatter = []
    iota_np = _iota_np()
    for c in range(NCORES):
        hs = np.zeros((P, VT_PER_CORE * K * D), dtype=np.float16)
        xr = np.full((P, VT_PER_CORE * K), -1000.0, dtype=np.float32)
        seq = list(range(c * VT_PER_CORE, (c + 1) * VT_PER_CORE))
        for vt, g in enumerate(seq):
            s, e = int(starts[g]), int(starts[g + 1])
            cnt = e - s
            block = np.zeros((K * P, D), dtype=np.float16)
            block[:cnt] = H[order[s:e]].astype(np.float16)
            hs[:, vt * K * D : (vt + 1) * K * D] = _tilemajor(block, K)
            xv = np.full(K * P, -1000.0, dtype=np.float32)
            xv[:cnt] = (Xs[s:e] - g * P).astype(np.float32)
            xr[:, vt * K : (vt + 1) * K] = xv.reshape(K, P).T
        in_maps.append({"hs": _pack_hs(xr, iota_np, hs)})
        scatter.append(seq)
    return in_maps, scatter


def kernel(H, X_neis, V=V):
    global LAST_EXEC_NS, LAST_RESULTS
    H = np.asarray(H, dtype=np.float32)
    X = np.asarray(X_neis).astype(np.int64)
    assert H.shape == (N, D) and X.shape == (N,)

    order = np.argsort(X, kind="stable")
    Xs = X[order]
    counts = np.bincount(X, minlength=V).reshape(NVT, P).sum(axis=1)
    starts = np.zeros(NVT + 1, dtype=np.int64)
    np.cumsum(counts, out=starts[1:])

    groups = _partition_tiles(counts, 0, 128)
    if groups is not None:
        mode, K = "B5", 5
    else:
        groups = _partition_tiles(counts, -128, 128)
        if groups is not None:
            mode, K = "B6", 6
    if groups is not None:
        in_maps, scatter = _shard_mode_b(H, order, Xs, starts, groups, mode, K)
    else:
        mode, K = "A", max(1, int(-(-counts.max() // P)))
        in_maps, scatter = _shard_mode_a(H, order, Xs, starts, K)

    key = (mode, K)
    if key not in _PROGRAM_CACHE:
        _PROGRAM_CACHE[key] = _build_program(mode, K)
    nc = _PROGRAM_CACHE[key]

    try:
        res = run_bass_kernel_spmd(nc, in_maps, list(range(NCORES)), trace=TRACE)
    except Exception:
        # transient NRT/device hiccups have been observed; retry once
        res = run_bass_kernel_spmd(nc, in_maps, list(range(NCORES)), trace=TRACE)
    LAST_EXEC_NS = res.exec_time_ns
    LAST_RESULTS = res

    full = np.empty((V, D), dtype=np.float32)
    for c in range(NCORES):
        o = np.asarray(res.results[c]["out"], dtype=np.float32)  # [P, VT*D]
        for vt, g in enumerate(scatter[c]):
            full[g * P : (g + 1) * P] = o[:, vt * D : (vt + 1) * D]
    return full


# revision 45
# speedup vs baseline: 1.0803x; 1.0803x over previous
"""Segment-sum (AggrSum) kernel for 8 Trainium2 NeuronCores.

Math: out[v, :] = sum_{n: X_neis[n] == v} H[n, :]   (H [N, D], out [V, D])

Strategy (V-sharding with host-side bucketing as the sharding step):
  - Sort edge ids by target vocab index; group edges by 128-row vocab tile.
  - Partition the 64 vocab tiles into 8 balanced groups of 8 (one per
    core), ordered inside each group so that packed prefix drift stays
    within [0, 128] rows of 512*vt ("mode B5"). Each core reads an
    exactly-packed edge stream; every vocab tile's edges are covered by
    a fixed window of K=5 physical 128-row tiles at static offsets, and
    the one-hot masks zero out foreign rows.
  - H rows are uploaded as a single fp16 plane (rel err ~3e-4, well
    under the 2e-2 gate); one fp16 matmul per (vt, k) window tile
    accumulates into a [128, 256] fp32 PSUM tile.  Mask emission
    (one DVE is_equal per vocab tile) is split across the Vector and
    GpSimd engines so neither gates the DMA-bound stream.  PSUM->SBUF
    copies run on the Scalar (ACT) engine with an fp32->fp16 convert;
    outputs stream back as fp16 and the host upconverts + scatters.
  - Fallbacks: drift in [-128,128] with K=6 windows ("mode B6"), then
    padded per-vt tiles ("mode A") for pathological inputs.
"""

import numpy as np

import concourse.bacc as bacc
import concourse.mybir as mybir
import concourse.tile as tile
from concourse.bass_utils import run_bass_kernel_spmd

N, D, V = 32768, 256, 8192
NCORES = 8
P = 128
VT_PER_CORE = V // P // NCORES  # 8 vocab tiles of 128 per core
NVT = V // P  # 64 global vocab tiles
NTILES_B = 33  # physical 128-row tiles per core in mode B (K=5 and K=6)
# Free-dim-512 dummies, ~427ns each at the cold 1.2 GHz clock.  The HAM
# grant needs one FULL free-running 4096-cycle window of activity: 7 dummies
# (3.39us) measurably fails to trigger it, 8 (3.82us) works.  13 bridges the
# whole preamble-to-chunk1 window (the early real matmuls are DMA-gated
# anyway), so the PE is guaranteed warm and insensitive to window phase.
N_WARMUP = 13

TRACE = False
LAST_EXEC_NS = None
LAST_RESULTS = None

_PROGRAM_CACHE: dict = {}

def _proc_order(mode: str) -> list:
    """vt processing order: B5 runs vt 7 (whose window is pulled to
    mid-stream) early and leaves the stream-tail owner vt 6 for last."""
    if mode == "B5":
        return [0, 1, 7, 2, 3, 4, 5, 6]
    return list(range(VT_PER_CORE))


def _win_lo(mode: str, K: int, vt: int) -> int:
    """First physical tile of vocab tile vt's window."""
    if mode == "B5":
        return 4 * vt
    if mode == "B6":
        return 0 if vt == 0 else 4 * vt - 1
    return vt * K  # mode A: padded, disjoint windows


def _build_common(nc, tc, pools, mode, K, n_phys_tiles, hs, out, chunk_tiles):
    f32 = mybir.dt.float32
    f16 = mybir.dt.float16
    hpool, mv_pool, opool, psum_pool, warm_pool = pools
    nconst = VT_PER_CORE * K + P
    iota_off = VT_PER_CORE * K

    # Warm up the PE's HAM clock gate (throttled 1.2 GHz until ~3.4us of
    # sustained matmul activity).  Long free-dim dummies on a memset scratch
    # tile span the whole preamble-to-first-data window (~3.4us) without a
    # DMA dependency, so PE activity is continuous from the first possible
    # cycle and the 2.4 GHz grant lands just as the real stream ramps up.
    warm_sb = warm_pool.tile([P, 4 * P], f16, name="warm_sb", tag="warmsb")
    nc.gpsimd.memset(warm_sb[:], 0.0)
    warm_ps = psum_pool.tile([P, 4 * P], f32, name="warm", tag="warm", bufs=1)
    for _ in range(N_WARMUP):
        nc.tensor.matmul(
            out=warm_ps[:],
            lhsT=warm_sb[:, :P],
            rhs=warm_sb[:],
            start=True,
            stop=True,
        )

    # Input stream entirely on Sync's HWDGE ring.  The mask operands
    # (consts) ride in front of the first chunk — one DMA, one semaphore;
    # a separate consts DMA would delay every later chunk by its ~0.65us
    # descriptor-generation slot for no gain (the first matmuls are gated
    # by chunk 0's completion either way).
    chunks = []
    t0 = 0
    first = True
    const_sb = None
    for ct in chunk_tiles:
        if first:
            ch = hpool.tile([P, nconst + ct * D], f16, name="ch0")
            nc.sync.dma_start(ch[:], hs[:, : nconst + ct * D])
            const_sb = ch[:, :nconst]
            chunks.append((t0, ct, ch, nconst))
            first = False
        else:
            ch = hpool.tile([P, ct * D], f16, name="ch")
            nc.sync.dma_start(
                ch[:], hs[:, nconst + t0 * D : nconst + (t0 + ct) * D]
            )
            chunks.append((t0, ct, ch, 0))
        t0 += ct
    assert t0 == n_phys_tiles

    def stream_pos(t):
        # B modes stream tile 32 right after the consts; B5 additionally
        # pulls the stream-tail vt's remaining window (tiles 28-31) to
        # mid-stream so only tile 27 arrives last.
        if mode == "B5":
            if t == NTILES_B - 1:
                return 0
            if t <= 8:
                return t + 1
            if t >= 28:
                return 10 + (t - 28)
            return t + 5
        if mode == "B6":
            return 0 if t == NTILES_B - 1 else t + 1
        return t

    def rhs_slice(t):
        p = stream_pos(t)
        for c0, ct, ch, off0 in chunks:
            if c0 <= p < c0 + ct:
                off = off0 + (p - c0) * D
                return ch[:, off : off + D]
        raise AssertionError(t)

    # Mask emission on Vector (TRN2's Pool slot rejects TENSOR_TENSOR).
    # One slice-written buffer; subtile deps keep the matmuls fine-grained.
    big_m = mv_pool.tile([P, VT_PER_CORE * K * P], f16, name="big_m", bufs=1)

    def emit_mask(vt, k0, k1):
        # masks k0..k1-1 for vocab tile vt in one DVE op:
        # m[p, k, q] = (xrel[p, vt*K+k] == iota[q])
        nk = k1 - k0
        m = big_m[:, (vt * K + k0) * P : (vt * K + k1) * P]
        nc.vector.tensor_tensor(
            out=m.rearrange("p (k q) -> p k q", k=nk),
            in0=const_sb[:, vt * K + k0 : vt * K + k1]
            .unsqueeze(2)
            .broadcast_to([P, nk, P]),
            in1=const_sb[:, iota_off : iota_off + P]
            .unsqueeze(1)
            .broadcast_to([P, nk, P]),
            op=mybir.AluOpType.is_equal,
        )

    # Processing order: in B5 the prefetched vt (7) runs early; vt 6 owns
    # the stream tail and is processed last with only one matmul left
    # after the final streamed tile lands.
    proc = _proc_order(mode)

    # First mask column alone so the PE can start the moment chunk 0 lands;
    # then the rest of the first vt, then one op per remaining vt, in
    # processing order.
    emit_mask(proc[0], 0, 1)
    emit_mask(proc[0], 1, K)
    for vt in proc[1:]:
        emit_mask(vt, 0, K)

    big_ot = opool.tile([P, VT_PER_CORE * D], f16, name="big_ot", bufs=1)

    # psum->sbuf fp16 copies: first six processed vts on Scalar (while
    # Vector is emitting masks), the last two on Vector once it frees up.
    # Write-backs go out on Sync's ring, which has drained its input
    # descriptors by then: one DMA for the first six slots, one for the
    # final two.
    for slot, vt in enumerate(proc):
        ps = psum_pool.tile([P, D], f32, name="ps")
        # The accumulation order within a vt is free; consume the
        # early-prefetched tile before the stream-tail tile so only the
        # final streamed tile's matmul remains after the stream ends.
        if mode == "B5" and vt == 6:
            k_order = [0, 1, 2, 4, 3]
        elif mode == "B6" and vt == VT_PER_CORE - 1:
            k_order = [0, K - 1] + list(range(1, K - 1))
        else:
            k_order = list(range(K))
        for i, k in enumerate(k_order):
            t = _win_lo(mode, K, vt) + k
            nc.tensor.matmul(
                out=ps[:],
                lhsT=big_m[:, (vt * K + k) * P : (vt * K + k + 1) * P],
                rhs=rhs_slice(t),
                start=(i == 0),
                stop=(i == len(k_order) - 1),
            )
        ot = big_ot[:, slot * D : (slot + 1) * D]
        if slot < 7:
            nc.scalar.copy(ot, ps[:])
        else:
            nc.vector.tensor_copy(ot, ps[:])
        # Staged write-backs on Scalar's ring, issued as soon as their
        # slots' copies land (all post-input-stream, so no read/write ring
        # contention): the earlier groups' transfers overlap the matmul /
        # copy tail, and the final group is a single small (64KB) write so
        # the last completion receipt comes as early as possible.  (Sync's
        # ring measured a ~1.3us first-byte penalty for late writes — stay
        # on Scalar's.)
        for g0, g1 in ((0, 3), (3, 5), (5, 7), (7, 8)):
            if slot == g1 - 1:
                nc.scalar.dma_start(
                    out[:, g0 * D : g1 * D], big_ot[:, g0 * D : g1 * D]
                )


def _build_program(mode, K):
    """mode 'B5'/'B6': exact-packed windows; mode 'A': padded (K tiles/vt)."""
    f16 = mybir.dt.float16
    if mode == "B5":
        n_phys = NTILES_B
        # stream order: [t32,t0], [t1-8], [t28-31], [t9-16], [t17-24],
        # [t25,26], [t27] — vt7's window lands mid-stream, tile 27 last
        chunk_tiles = [2, 8, 4, 8, 8, 2, 1]
    elif mode == "B6":
        n_phys = NTILES_B
        chunk_tiles = [2, 8, 8, 8, 4, 2, 1]
    else:
        n_phys = VT_PER_CORE * K
        nt = n_phys
        chunk_tiles = []
        while nt > 0:
            chunk_tiles.append(min(7, nt))
            nt -= min(7, nt)
    nconst = VT_PER_CORE * K + P

    nc = bacc.Bacc("TRN2", target_bir_lowering=False)
    hs = nc.dram_tensor("hs", [P, nconst + n_phys * D], f16, kind="ExternalInput")
    out = nc.dram_tensor("out", [P, VT_PER_CORE * D], f16, kind="ExternalOutput")

    with tile.TileContext(nc) as tc:
        with (
            tc.tile_pool(name="h", bufs=min(len(chunk_tiles), 16)) as hpool,
            tc.tile_pool(name="mv", bufs=1) as mv_pool,
            tc.tile_pool(name="o", bufs=1) as opool,
            tc.tile_pool(name="warm", bufs=1) as warm_pool,
            tc.tile_pool(name="psum", bufs=7, space="PSUM") as psum_pool,
        ):
            _build_common(
                nc,
                tc,
                (hpool, mv_pool, opool, psum_pool, warm_pool),
                mode,
                K,
                n_phys,
                hs,
                out,
                chunk_tiles,
            )
    nc.finalize()
    return nc


def _order_group(counts, tiles, lo, hi):
    """Order `tiles` so prefix drift (run - 512*k) stays in [lo, hi] at
    every interior step and <= hi at the end.  DFS, largest-first."""
    tiles = sorted(tiles, key=lambda g: -counts[g])
    n = len(tiles)
    used = [False] * n
    seq = []

    def dfs(k, run):
        if k == n:
            return True
        prev = None
        for i in range(n):
            if used[i]:
                continue
            c = int(counts[tiles[i]])
            if c == prev:
                continue  # identical count -> identical subtree
            prev = c
            d = run + c - 512 * (k + 1)
            if d > hi:
                continue
            if k + 1 < n and d < lo:
                continue
            used[i] = True
            seq.append(tiles[i])
            if dfs(k + 1, run + c):
                return True
            used[i] = False
            seq.pop()
        return False

    return list(seq) if dfs(0, 0) else None


def _partition_tiles(counts, lo, hi):
    """Partition the 64 vocab tiles into 8 groups of 8, each ordered so
    packed prefix drift stays in [lo, hi].  Returns list of per-core
    sequences of global tile ids, or None."""
    rng = np.random.RandomState(0)
    base = np.argsort(counts)[::-1]
    for attempt in range(40):
        if attempt == 0:
            order = base
        else:
            order = rng.permutation(NVT)
            order = order[np.argsort(counts[order])[::-1]]
        groups = [[] for _ in range(NCORES)]
        for i, g in enumerate(order):
            rnd, pos = divmod(i, NCORES)
            c = pos if rnd % 2 == 0 else NCORES - 1 - pos
            groups[c].append(int(g))
        seqs = []
        for c in range(NCORES):
            seq = _order_group(counts, groups[c], lo, hi)
            if seq is None:
                break
            seqs.append(seq)
        if len(seqs) == NCORES:
            return seqs
    return None


def _iota_np():
    return np.tile(np.arange(P, dtype=np.float32), (P, 1))


def _pack_hs(xr, iota_np, hs_tiles):
    """consts ([P, nk] xrel + [P, 128] iota) prepended to the tile-major
    H stream -> single [P, nconst + ntiles*D] fp16 input."""
    return np.hstack([xr, iota_np, hs_tiles]).astype(np.float16)


def _tilemajor(block_f16, ntiles):
    """[ntiles*P, D] fp16 -> [P, ntiles*D] tile-major."""
    return (
        block_f16.reshape(ntiles, P, D).transpose(1, 0, 2).reshape(P, ntiles * D)
    )


def _shard_mode_b(H, order, Xs, starts, groups, mode, K):
    in_maps = []
    scatter = []
    iota_np = _iota_np()
    for c in range(NCORES):
        seq = groups[c]
        rows = np.concatenate([order[starts[g] : starts[g + 1]] for g in seq])
        xval = np.concatenate([Xs[starts[g] : starts[g + 1]] for g in seq]).astype(
            np.float64
        )
        n_c = len(rows)
        block = np.zeros((NTILES_B * P, D), dtype=np.float16)
        block[:n_c] = H[rows].astype(np.float16)
        xpad = np.full(NTILES_B * P, -1000.0, dtype=np.float64)
        xpad[:n_c] = xval
        # stream tile order must match the device-side stream_pos mapping
        if mode == "B5":
            perm = [32, 0] + list(range(1, 9)) + list(range(28, 32)) + list(
                range(9, 28)
            )
        else:
            perm = [32] + list(range(32))
        stream = np.concatenate([block[t * P : (t + 1) * P] for t in perm])
        hs_tiles = _tilemajor(stream, NTILES_B)
        xr = np.full((P, VT_PER_CORE * K), -1000.0, dtype=np.float32)
        for vt in range(VT_PER_CORE):
            base = 128.0 * seq[vt]
            for k in range(K):
                t = _win_lo(mode, K, vt) + k
                xr[:, vt * K + k] = (xpad[t * P : (t + 1) * P] - base).astype(
                    np.float32
                )
        in_maps.append({"hs": _pack_hs(xr, iota_np, hs_tiles)})
        # output slots are in processing order
        scatter.append([seq[vt] for vt in _proc_order(mode)])
    return in_maps, scatter


def _shard_mode_a(H, order, Xs, starts, K):
    in_maps = []
    scatter = []
    iota_np = _iota_np()
    for c in range(NCORES):
        hs = np.zeros((P, VT_PER_CORE * K * D), dtype=np.float16)
        xr = np.full((P, VT_PER_CORE * K), -1000.0, dtype=np.float32)
        seq = list(range(c * VT_PER_CORE, (c + 1) * VT_PER_CORE))
        for vt, g in enumerate(seq):
            s, e = int(starts[g]), int(starts[g + 1])
            cnt = e - s
            block = np.zeros((K * P, D), dtype=np.float16)
            block[:cnt] = H[order[s:e]].astype(np.float16)
            hs[:, vt * K * D : (vt + 1) * K * D] = _tilemajor(block, K)
            xv = np.full(K * P, -1000.0, dtype=np.float32)
            xv[:cnt] = (Xs[s:e] - g * P).astype(np.float32)
            xr[:, vt * K : (vt + 1) * K] = xv.reshape(K, P).T
        in_maps.append({"hs": _pack_hs(xr, iota_np, hs)})
        scatter.append(seq)
    return in_maps, scatter


def kernel(H, X_neis, V=V):
    global LAST_EXEC_NS, LAST_RESULTS
    H = np.asarray(H, dtype=np.float32)
    X = np.asarray(X_neis).astype(np.int64)
    assert H.shape == (N, D) and X.shape == (N,)

    order = np.argsort(X, kind="stable")
    Xs = X[order]
    counts = np.bincount(X, minlength=V).reshape(NVT, P).sum(axis=1)
    starts = np.zeros(NVT + 1, dtype=np.int64)
    np.cumsum(counts, out=starts[1:])

    groups = _partition_tiles(counts, 0, 128)
    if groups is not None:
        mode, K = "B5", 5
    else:
        groups = _partition_tiles(counts, -128, 128)
        if groups is not None:
            mode, K = "B6", 6
    if groups is not None:
        in_maps, scatter = _shard_mode_b(H, order, Xs, starts, groups, mode, K)
    else:
        mode, K = "A", max(1, int(-(-counts.max() // P)))
        in_maps, scatter = _shard_mode_a(H, order, Xs, starts, K)

    key = (mode, K)
    if key not in _PROGRAM_CACHE:
        _PROGRAM_CACHE[key] = _build_program(mode, K)
    nc = _PROGRAM_CACHE[key]

    try:
        res = run_bass_kernel_spmd(nc, in_maps, list(range(NCORES)), trace=TRACE)
    except Exception:
        # transient NRT/device hiccups have been observed; retry once
        res = run_bass_kernel_spmd(nc, in_maps, list(range(NCORES)), trace=TRACE)
    LAST_EXEC_NS = res.exec_time_ns
    LAST_RESULTS = res

    full = np.empty((V, D), dtype=np.float32)
    for c in range(NCORES):
        o = np.asarray(res.results[c]["out"], dtype=np.float32)  # [P, VT*D]
        for vt, g in enumerate(scatter[c]):
            full[g * P : (g + 1) * P] = o[:, vt * D : (vt + 1) * D]
    return full


# revision 46
# speedup vs baseline: 1.1257x; 1.0421x over previous
"""Segment-sum (AggrSum) kernel for 8 Trainium2 NeuronCores.

Math: out[v, :] = sum_{n: X_neis[n] == v} H[n, :]   (H [N, D], out [V, D])

Strategy (V-sharding with host-side bucketing as the sharding step):
  - Sort edge ids by target vocab index; group edges by 128-row vocab tile.
  - Partition the 64 vocab tiles into 8 balanced groups of 8 (one per
    core), ordered inside each group so that packed prefix drift stays
    within [0, 128] rows of 512*vt ("mode B5"). Each core reads an
    exactly-packed edge stream; every vocab tile's edges are covered by
    a fixed window of K=5 physical 128-row tiles at static offsets, and
    the one-hot masks zero out foreign rows.
  - H rows are uploaded as a single fp16 plane (rel err ~3e-4, well
    under the 2e-2 gate); one fp16 matmul per (vt, k) window tile
    accumulates into a [128, 256] fp32 PSUM tile.  Mask emission
    (one DVE is_equal per vocab tile) is split across the Vector and
    GpSimd engines so neither gates the DMA-bound stream.  PSUM->SBUF
    copies run on the Scalar (ACT) engine with an fp32->fp16 convert;
    outputs stream back as fp16 and the host upconverts + scatters.
  - Fallbacks: drift in [-128,128] with K=6 windows ("mode B6"), then
    padded per-vt tiles ("mode A") for pathological inputs.
"""

import numpy as np

import concourse.bacc as bacc
import concourse.mybir as mybir
import concourse.tile as tile
from concourse.bass_utils import run_bass_kernel_spmd

N, D, V = 32768, 256, 8192
NCORES = 8
P = 128
VT_PER_CORE = V // P // NCORES  # 8 vocab tiles of 128 per core
NVT = V // P  # 64 global vocab tiles
NTILES_B = 33  # physical 128-row tiles per core in mode B (K=5 and K=6)
# Free-dim-512 dummies, ~427ns each at the cold 1.2 GHz clock.  The HAM
# grant needs one FULL free-running 4096-cycle window of activity: 7 dummies
# (3.39us) measurably fails to trigger it, 8 (3.82us) works.  13 bridges the
# whole preamble-to-chunk1 window (the early real matmuls are DMA-gated
# anyway), so the PE is guaranteed warm and insensitive to window phase.
N_WARMUP = 13

TRACE = False
LAST_EXEC_NS = None
LAST_RESULTS = None

_PROGRAM_CACHE: dict = {}

def _proc_order(mode: str) -> list:
    """vt processing order: B5 runs vt 7 (whose window is pulled to
    mid-stream) early and leaves the stream-tail owner vt 6 for last."""
    if mode == "B5":
        return [0, 1, 7, 2, 3, 4, 5, 6]
    return list(range(VT_PER_CORE))


def _win_lo(mode: str, K: int, vt: int) -> int:
    """First physical tile of vocab tile vt's window."""
    if mode == "B5":
        return 4 * vt
    if mode == "B6":
        return 0 if vt == 0 else 4 * vt - 1
    return vt * K  # mode A: padded, disjoint windows


def _build_common(nc, tc, pools, mode, K, n_phys_tiles, hs, out, chunk_tiles):
    f32 = mybir.dt.float32
    f16 = mybir.dt.float16
    hpool, mv_pool, opool, psum_pool, warm_pool = pools
    nconst = VT_PER_CORE * K + P
    iota_off = VT_PER_CORE * K

    # Warm up the PE's HAM clock gate (throttled 1.2 GHz until ~3.4us of
    # sustained matmul activity).  Long free-dim dummies on a memset scratch
    # tile span the whole preamble-to-first-data window (~3.4us) without a
    # DMA dependency, so PE activity is continuous from the first possible
    # cycle and the 2.4 GHz grant lands just as the real stream ramps up.
    warm_sb = warm_pool.tile([P, 4 * P], f16, name="warm_sb", tag="warmsb")
    nc.gpsimd.memset(warm_sb[:], 0.0)
    warm_ps = psum_pool.tile([P, 4 * P], f32, name="warm", tag="warm", bufs=1)
    for _ in range(N_WARMUP):
        nc.tensor.matmul(
            out=warm_ps[:],
            lhsT=warm_sb[:, :P],
            rhs=warm_sb[:],
            start=True,
            stop=True,
        )

    # Input stream entirely on Sync's HWDGE ring.  The mask operands
    # (consts) ride in front of the first chunk — one DMA, one semaphore;
    # a separate consts DMA would delay every later chunk by its ~0.65us
    # descriptor-generation slot for no gain (the first matmuls are gated
    # by chunk 0's completion either way).
    chunks = []
    t0 = 0
    first = True
    const_sb = None
    for ct in chunk_tiles:
        if first:
            ch = hpool.tile([P, nconst + ct * D], f16, name="ch0")
            nc.sync.dma_start(ch[:], hs[:, : nconst + ct * D])
            const_sb = ch[:, :nconst]
            chunks.append((t0, ct, ch, nconst))
            first = False
        else:
            ch = hpool.tile([P, ct * D], f16, name="ch")
            nc.sync.dma_start(
                ch[:], hs[:, nconst + t0 * D : nconst + (t0 + ct) * D]
            )
            chunks.append((t0, ct, ch, 0))
        t0 += ct
    assert t0 == n_phys_tiles

    def stream_pos(t):
        # B modes stream tile 32 right after the consts; B5 additionally
        # pulls the stream-tail vt's remaining window (tiles 28-31) to
        # mid-stream so only tile 27 arrives last.
        if mode == "B5":
            if t == NTILES_B - 1:
                return 0
            if t <= 8:
                return t + 1
            if t >= 28:
                return 10 + (t - 28)
            return t + 5
        if mode == "B6":
            return 0 if t == NTILES_B - 1 else t + 1
        return t

    def rhs_slice(t):
        p = stream_pos(t)
        for c0, ct, ch, off0 in chunks:
            if c0 <= p < c0 + ct:
                off = off0 + (p - c0) * D
                return ch[:, off : off + D]
        raise AssertionError(t)

    # Mask emission on Vector (TRN2's Pool slot rejects TENSOR_TENSOR).
    # One slice-written buffer; subtile deps keep the matmuls fine-grained.
    big_m = mv_pool.tile([P, VT_PER_CORE * K * P], f16, name="big_m", bufs=1)

    def emit_mask(vt, k0, k1):
        # masks k0..k1-1 for vocab tile vt in one DVE op:
        # m[p, k, q] = (xrel[p, vt*K+k] == iota[q])
        nk = k1 - k0
        m = big_m[:, (vt * K + k0) * P : (vt * K + k1) * P]
        nc.vector.tensor_tensor(
            out=m.rearrange("p (k q) -> p k q", k=nk),
            in0=const_sb[:, vt * K + k0 : vt * K + k1]
            .unsqueeze(2)
            .broadcast_to([P, nk, P]),
            in1=const_sb[:, iota_off : iota_off + P]
            .unsqueeze(1)
            .broadcast_to([P, nk, P]),
            op=mybir.AluOpType.is_equal,
        )

    # Processing order: in B5 the prefetched vt (7) runs early; vt 6 owns
    # the stream tail and is processed last with only one matmul left
    # after the final streamed tile lands.
    proc = _proc_order(mode)

    # First mask column alone so the PE can start the moment chunk 0 lands;
    # then the rest of the first vt, then one op per remaining vt, in
    # processing order.
    emit_mask(proc[0], 0, 1)
    emit_mask(proc[0], 1, K)
    for vt in proc[1:]:
        emit_mask(vt, 0, K)

    big_ot = opool.tile([P, VT_PER_CORE * D], f16, name="big_ot", bufs=1)

    # psum->sbuf fp16 copies: first six processed vts on Scalar (while
    # Vector is emitting masks), the last two on Vector once it frees up.
    # Write-backs go out on Sync's ring, which has drained its input
    # descriptors by then: one DMA for the first six slots, one for the
    # final two.
    for slot, vt in enumerate(proc):
        ps = psum_pool.tile([P, D], f32, name="ps")
        # The accumulation order within a vt is free; consume the
        # early-prefetched tile before the stream-tail tile so only the
        # final streamed tile's matmul remains after the stream ends.
        if mode == "B5" and vt == 6:
            k_order = [0, 1, 2, 4, 3]
        elif mode == "B6" and vt == VT_PER_CORE - 1:
            k_order = [0, K - 1] + list(range(1, K - 1))
        else:
            k_order = list(range(K))
        for i, k in enumerate(k_order):
            t = _win_lo(mode, K, vt) + k
            nc.tensor.matmul(
                out=ps[:],
                lhsT=big_m[:, (vt * K + k) * P : (vt * K + k + 1) * P],
                rhs=rhs_slice(t),
                start=(i == 0),
                stop=(i == len(k_order) - 1),
            )
        ot = big_ot[:, slot * D : (slot + 1) * D]
        if slot < 7:
            nc.scalar.copy(ot, ps[:])
        else:
            nc.vector.tensor_copy(ot, ps[:])
        # Write-backs on Scalar's ring (Sync's measured a ~1.3us
        # first-byte penalty for late writes): seven slots in one DMA once
        # their copies land, then the stream-tail vt alone as a small
        # (64KB) trailing write so the final completion receipt comes as
        # early as possible.
        for g0, g1 in ((0, 7), (7, 8)):
            if slot == g1 - 1:
                nc.scalar.dma_start(
                    out[:, g0 * D : g1 * D], big_ot[:, g0 * D : g1 * D]
                )


def _build_program(mode, K):
    """mode 'B5'/'B6': exact-packed windows; mode 'A': padded (K tiles/vt)."""
    f16 = mybir.dt.float16
    if mode == "B5":
        n_phys = NTILES_B
        # stream order: [t32,t0], [t1-8], [t28-31], [t9-16], [t17-24],
        # [t25,26], [t27] — vt7's window lands mid-stream, tile 27 last
        chunk_tiles = [2, 8, 4, 8, 8, 2, 1]
    elif mode == "B6":
        n_phys = NTILES_B
        chunk_tiles = [2, 8, 8, 8, 4, 2, 1]
    else:
        n_phys = VT_PER_CORE * K
        nt = n_phys
        chunk_tiles = []
        while nt > 0:
            chunk_tiles.append(min(7, nt))
            nt -= min(7, nt)
    nconst = VT_PER_CORE * K + P

    nc = bacc.Bacc("TRN2", target_bir_lowering=False)
    hs = nc.dram_tensor("hs", [P, nconst + n_phys * D], f16, kind="ExternalInput")
    out = nc.dram_tensor("out", [P, VT_PER_CORE * D], f16, kind="ExternalOutput")

    with tile.TileContext(nc) as tc:
        with (
            tc.tile_pool(name="h", bufs=min(len(chunk_tiles), 16)) as hpool,
            tc.tile_pool(name="mv", bufs=1) as mv_pool,
            tc.tile_pool(name="o", bufs=1) as opool,
            tc.tile_pool(name="warm", bufs=1) as warm_pool,
            tc.tile_pool(name="psum", bufs=7, space="PSUM") as psum_pool,
        ):
            _build_common(
                nc,
                tc,
                (hpool, mv_pool, opool, psum_pool, warm_pool),
                mode,
                K,
                n_phys,
                hs,
                out,
                chunk_tiles,
            )
    nc.finalize()
    return nc


def _order_group(counts, tiles, lo, hi):
    """Order `tiles` so prefix drift (run - 512*k) stays in [lo, hi] at
    every interior step and <= hi at the end.  DFS, largest-first."""
    tiles = sorted(tiles, key=lambda g: -counts[g])
    n = len(tiles)
    used = [False] * n
    seq = []

    def dfs(k, run):
        if k == n:
            return True
        prev = None
        for i in range(n):
            if used[i]:
                continue
            c = int(counts[tiles[i]])
            if c == prev:
                continue  # identical count -> identical subtree
            prev = c
            d = run + c - 512 * (k + 1)
            if d > hi:
                continue
            if k + 1 < n and d < lo:
                continue
            used[i] = True
            seq.append(tiles[i])
            if dfs(k + 1, run + c):
                return True
            used[i] = False
            seq.pop()
        return False

    return list(seq) if dfs(0, 0) else None


def _partition_tiles(counts, lo, hi):
    """Partition the 64 vocab tiles into 8 groups of 8, each ordered so
    packed prefix drift stays in [lo, hi].  Returns list of per-core
    sequences of global tile ids, or None."""
    rng = np.random.RandomState(0)
    base = np.argsort(counts)[::-1]
    for attempt in range(40):
        if attempt == 0:
            order = base
        else:
            order = rng.permutation(NVT)
            order = order[np.argsort(counts[order])[::-1]]
        groups = [[] for _ in range(NCORES)]
        for i, g in enumerate(order):
            rnd, pos = divmod(i, NCORES)
            c = pos if rnd % 2 == 0 else NCORES - 1 - pos
            groups[c].append(int(g))
        seqs = []
        for c in range(NCORES):
            seq = _order_group(counts, groups[c], lo, hi)
            if seq is None:
                break
            seqs.append(seq)
        if len(seqs) == NCORES:
            return seqs
    return None


def _iota_np():
    return np.tile(np.arange(P, dtype=np.float32), (P, 1))


def _pack_hs(xr, iota_np, hs_tiles):
    """consts ([P, nk] xrel + [P, 128] iota) prepended to the tile-major
    H stream -> single [P, nconst + ntiles*D] fp16 input."""
    return np.hstack([xr, iota_np, hs_tiles]).astype(np.float16)


def _tilemajor(block_f16, ntiles):
    """[ntiles*P, D] fp16 -> [P, ntiles*D] tile-major."""
    return (
        block_f16.reshape(ntiles, P, D).transpose(1, 0, 2).reshape(P, ntiles * D)
    )


def _shard_mode_b(H, order, Xs, starts, groups, mode, K):
    in_maps = []
    scatter = []
    iota_np = _iota_np()
    for c in range(NCORES):
        seq = groups[c]
        rows = np.concatenate([order[starts[g] : starts[g + 1]] for g in seq])
        xval = np.concatenate([Xs[starts[g] : starts[g + 1]] for g in seq]).astype(
            np.float64
        )
        n_c = len(rows)
        block = np.zeros((NTILES_B * P, D), dtype=np.float16)
        block[:n_c] = H[rows].astype(np.float16)
        xpad = np.full(NTILES_B * P, -1000.0, dtype=np.float64)
        xpad[:n_c] = xval
        # stream tile order must match the device-side stream_pos mapping
        if mode == "B5":
            perm = [32, 0] + list(range(1, 9)) + list(range(28, 32)) + list(
                range(9, 28)
            )
        else:
            perm = [32] + list(range(32))
        stream = np.concatenate([block[t * P : (t + 1) * P] for t in perm])
        hs_tiles = _tilemajor(stream, NTILES_B)
        xr = np.full((P, VT_PER_CORE * K), -1000.0, dtype=np.float32)
        for vt in range(VT_PER_CORE):
            base = 128.0 * seq[vt]
            for k in range(K):
                t = _win_lo(mode, K, vt) + k
                xr[:, vt * K + k] = (xpad[t * P : (t + 1) * P] - base).astype(
                    np.float32
                )
        in_maps.append({"hs": _pack_hs(xr, iota_np, hs_tiles)})
        # output slots are in processing order
        scatter.append([seq[vt] for vt in _proc_order(mode)])
    return in_maps, scatter


def _shard_mode_a(H, order, Xs, starts, K):
    in_maps = []
    scatter = []
    iota_np = _iota_np()
    for c in range(NCORES):
        hs = np.zeros((P, VT_PER_CORE * K * D), dtype=np.float16)
        xr = np.full((P, VT_PER_CORE * K), -1000.0, dtype=np.float32)
        seq = list(range(c * VT_PER_CORE, (c + 1) * VT_PER_CORE))
        for vt, g in enumerate(seq):
            s, e = int(starts[g]), int(starts[g + 1])
            cnt = e - s
            block = np.zeros((K * P, D), dtype=np.float16)
            block[:cnt] = H[order[s:e]].astype(np.float16)
            hs[:, vt * K * D : (vt + 1) * K * D] = _tilemajor(block, K)
            xv = np.full(K * P, -1000.0, dtype=np.float32)
            xv[:cnt] = (Xs[s:e] - g * P).astype(np.float32)
            xr[:, vt * K : (vt + 1) * K] = xv.reshape(K, P).T
        in_maps.append({"hs": _pack_hs(xr, iota_np, hs)})
        scatter.append(seq)
    return in_maps, scatter


def kernel(H, X_neis, V=V):
    global LAST_EXEC_NS, LAST_RESULTS
    H = np.asarray(H, dtype=np.float32)
    X = np.asarray(X_neis).astype(np.int64)
    assert H.shape == (N, D) and X.shape == (N,)

    order = np.argsort(X, kind="stable")
    Xs = X[order]
    counts = np.bincount(X, minlength=V).reshape(NVT, P).sum(axis=1)
    starts = np.zeros(NVT + 1, dtype=np.int64)
    np.cumsum(counts, out=starts[1:])

    groups = _partition_tiles(counts, 0, 128)
    if groups is not None:
        mode, K = "B5", 5
    else:
        groups = _partition_tiles(counts, -128, 128)
        if groups is not None:
            mode, K = "B6", 6
    if groups is not None:
        in_maps, scatter = _shard_mode_b(H, order, Xs, starts, groups, mode, K)
    else:
        mode, K = "A", max(1, int(-(-counts.max() // P)))
        in_maps, scatter = _shard_mode_a(H, order, Xs, starts, K)

    key = (mode, K)
    if key not in _PROGRAM_CACHE:
        _PROGRAM_CACHE[key] = _build_program(mode, K)
    nc = _PROGRAM_CACHE[key]

    try:
        res = run_bass_kernel_spmd(nc, in_maps, list(range(NCORES)), trace=TRACE)
    except Exception:
        # transient NRT/device hiccups have been observed; retry once
        res = run_bass_kernel_spmd(nc, in_maps, list(range(NCORES)), trace=TRACE)
    LAST_EXEC_NS = res.exec_time_ns
    LAST_RESULTS = res

    full = np.empty((V, D), dtype=np.float32)
    for c in range(NCORES):
        o = np.asarray(res.results[c]["out"], dtype=np.float32)  # [P, VT*D]
        for vt, g in enumerate(scatter[c]):
            full[g * P : (g + 1) * P] = o[:, vt * D : (vt + 1) * D]
    return full
